# revision 8
# baseline (speedup 1.0000x reference)
"""Trainium2 8-core kernel for nn_Attention_88948772700322.

Reference computes (N=1024, B=4, C=1024, H=16, hd=64):
    qkv = x @ w_qkv.T                      [N,B,3C]
    q,k,v per (b,h); attn = softmax(q k^T / 8) v
    out = (attn.transpose(2,1,0,3)).reshape(N,B,C) @ w_proj.T + b_proj
The reshape interleaves H and B: proj-input channel c of output-batch bn is
attention head h = 4*bn + c//256, original batch b2 = (c%256)//64, dim d = c%64.

Sharding: tensor-parallel over heads — core i owns heads {2i, 2i+1}, all
batches/tokens (6.44 GFLOP/core, perfectly balanced).  Each core computes a
partial projection over its 512 proj-input channels for output batch bn=i//2;
host sums core pairs (the "all-reduce after proj" realized in unshard).

Host-side prep absorbs every layout nuisance:
  - xT [C, B*N] bf16, tokens batch-major  -> qkv needs no on-chip transpose
  - w_qk [C, 256] (cols q_h0,q_h1,k_h0,k_h1), q pre-scaled by 1/8
  - w_v  [C, 128] (cols v_h0,v_h1)
  - w_p  [512, 1024] = w_proj columns permuted to (b2, h_local, d) row order

On-chip per core (v2):
  - warm-up matmuls on memset data during the initial DMA wait (HAM ramp)
  - qk^T via PE (d-major), v via PE (token-major)
  - scores transposed (keys on partitions), exp without max-subtraction
  - av col-packed: both heads' 64-dim outputs in one PSUM bank via column
    tiling; softmax denominators via 4-up col-tiled ones-matmuls (4
    concurrent streams per kc pair)
  - normalization with zero DMAs: denominator pair-sum + partition-broadcast
    by one PE matmul against a constant selection matrix, then DVE
    reciprocal + multiply
  - partial proj n-major with stationary reuse; fp16 output (host upcasts)
"""

import numpy as np
import ml_dtypes

import concourse.bass as bass
import concourse.mybir as mybir
from concourse import bacc
from concourse.tile import TileContext
from concourse.bass_utils import run_bass_kernel_spmd


N, B, C, H, HD = 1024, 4, 1024, 16, 64
NT = B * N          # 4096 tokens
NCORES = 8
BF = mybir.dt.bfloat16
F16 = mybir.dt.float16
F32 = mybir.dt.float32
bf16 = ml_dtypes.bfloat16

_NC_CACHE = {}


def build_nc():
    nc = bacc.Bacc()
    xT_e = nc.declare_dram_parameter("xT", [C, NT], BF, isOutput=False)
    wqk_e = nc.declare_dram_parameter("w_qk", [C, 256], BF, isOutput=False)
    wv_e = nc.declare_dram_parameter("w_v", [C, 128], BF, isOutput=False)
    wp_e = nc.declare_dram_parameter("w_p", [512, C], BF, isOutput=False)
    out_e = nc.declare_dram_parameter("out", [N, C], F16, isOutput=True)

    xT_ap = xT_e[:].rearrange("(co p) t -> p co t", p=128)    # [128, 8, 4096]
    wqk_ap = wqk_e[:].rearrange("(co p) m -> p co m", p=128)  # [128, 8, 256]
    wv_ap = wv_e[:].rearrange("(co p) m -> p co m", p=128)    # [128, 8, 128]
    wp_ap = wp_e[:].rearrange("(b2 p) d -> p b2 d", p=128)    # [128, 4, 1024]

    from contextlib import ExitStack
    with TileContext(nc) as tc:
        with ExitStack() as stk:
            cpool = stk.enter_context(tc.tile_pool(name="const", bufs=1))
            epool = stk.enter_context(tc.tile_pool(name="exp", bufs=6))
            spool = stk.enter_context(tc.tile_pool(name="small", bufs=4))
            opool = stk.enter_context(tc.tile_pool(name="outcp", bufs=6))
            attn_stk = ExitStack()
            # PSUM budget (8 banks): qkv 1, sT 2x2, av 2, den/bcast 1
            ps_qkv = attn_stk.enter_context(
                tc.tile_pool(name="ps_qkv", bufs=1, space="PSUM"))
            ps_sT = attn_stk.enter_context(
                tc.tile_pool(name="ps_sT", bufs=2, space="PSUM"))
            ps_av = attn_stk.enter_context(
                tc.tile_pool(name="ps_av", bufs=2, space="PSUM"))
            ps_den = attn_stk.enter_context(
                tc.tile_pool(name="ps_den", bufs=1, space="PSUM"))
            # ---- persistent SBUF tensors -------------------------------
            xc = [[cpool.tile([128, N], BF, name=f"xc_{b}_{kc}")
                   for kc in range(8)] for b in range(B)]
            wqk = cpool.tile([128, 8, 256], BF)
            wv = cpool.tile([128, 8, 128], BF)
            wp = cpool.tile([128, 4, C], BF)
            q_sb = cpool.tile([128, NT], BF)       # [ (h0|h1) d, token ]
            k_sb = cpool.tile([128, NT], BF)
            v_sb = cpool.tile([128, 32, 128], BF)  # [t_in, t_chunk, (hl d)]
            projin = cpool.tile([128, B, N], BF)   # [(hl,d), b2, n]
            ones1 = cpool.tile([128, 32], BF)      # denominator stationary
            smat = cpool.tile([128, 128], F16)     # den pair-sum + broadcast
            den_sb = cpool.tile([128, 512], F16)   # den staging (bcast rhs)
            junk = cpool.tile([128, 512], BF)      # PE warm-up operand

            # constants first: warm-up matmuls depend only on these memsets
            nc.vector.memset(junk[:], 0.5)
            nc.vector.memset(ones1[:], 1.0)
            nc.vector.memset(den_sb[:], 1.0)
            nc.vector.memset(smat[:], 0.0)
            # bcast row p of output <- den partials: hl0 from psum partitions
            # {0,32}, hl1 from {64,96}
            nc.vector.memset(smat[0:1, 0:64], 1.0)
            nc.vector.memset(smat[32:33, 0:64], 1.0)
            nc.vector.memset(smat[64:65, 64:128], 1.0)
            nc.vector.memset(smat[96:97, 64:128], 1.0)

            # ---- HAM warm-up: keep PE streaming while input DMAs land --
            jp = ps_qkv.tile([128, 512], F32, tag="qkv", name="jp")
            for i in range(12):
                nc.tensor.matmul(jp[:], junk[:, 0:128], junk[:],
                                 start=True, stop=True)

            # ---- input DMAs -------------------------------------------
            # weights on the Scalar HWDGE queue, x on Sync — the two issue
            # streams run in parallel so x transfers start ~2us earlier.
            nc.scalar.dma_start(out=wqk[:], in_=wqk_ap)
            nc.scalar.dma_start(out=wv[:], in_=wv_ap)
            nc.scalar.dma_start(out=wp[:], in_=wp_ap)
            for b in range(B):
                for kc in range(8):
                    if b == 0 and kc < 2:  # halves: first matmul starts early
                        for hh in range(2):
                            sl = slice(hh * 512, (hh + 1) * 512)
                            nc.sync.dma_start(out=xc[b][kc][:, sl],
                                              in_=xT_ap[:, kc, b * N:(b + 1) * N][:, sl])
                    else:
                        nc.sync.dma_start(out=xc[b][kc][:],
                                          in_=xT_ap[:, kc, b * N:(b + 1) * N])

            def qkv_block(b):
                # q,k d-major: stationary = wqk column block, moving = x
                for tc_i in (2 * b, 2 * b + 1):
                    j = tc_i - 2 * b
                    for which in range(2):  # 0=q, 1=k
                        ps = ps_qkv.tile([128, 512], F32, tag="qkv",
                                         name=f"qk_{b}_{tc_i}_{which}")
                        for kc in range(8):
                            nc.tensor.matmul(
                                ps[:], wqk[:, kc, which * 128:(which + 1) * 128],
                                xc[b][kc][:, j * 512:(j + 1) * 512],
                                start=(kc == 0), stop=(kc == 7))
                        dst = q_sb if which == 0 else k_sb
                        nc.vector.tensor_copy(
                            out=dst[:, tc_i * 512:(tc_i + 1) * 512], in_=ps[:])
                # v token-major: stationary = x token block, moving = wv
                for tt in range(8 * b, 8 * b + 8):
                    vps = ps_qkv.tile([128, 128], F32, tag="qkv",
                                      name=f"vps_{tt}")
                    for kc in range(8):
                        nc.tensor.matmul(vps[:],
                                         xc[b][kc][:, (tt - 8 * b) * 128:
                                                    (tt - 8 * b + 1) * 128],
                                         wv[:, kc, :],
                                         start=(kc == 0), stop=(kc == 7))
                    nc.vector.tensor_copy(out=v_sb[:, tt, :], in_=vps[:])

            def attn_block(b, qt):
                q_sl = slice(b * N + qt * 512, b * N + (qt + 1) * 512)
                av = ps_av.tile([128, 512], F32, tag="av", name=f"av_{b}_{qt}")
                den = ps_den.tile([128, 512], F32, tag="den",
                                  name=f"den_{b}_{qt}")
                es = []
                for kc in range(8):
                    k_sl = slice(b * N + kc * 128, b * N + (kc + 1) * 128)
                    sT = ps_sT.tile([128, 1024], F32, tag="sT",
                                    name=f"sT_{b}_{qt}_{kc}")
                    for hl in range(2):
                        nc.tensor.matmul(
                            sT[:, hl * 512:(hl + 1) * 512],
                            k_sb[hl * 64:(hl + 1) * 64, k_sl],
                            q_sb[hl * 64:(hl + 1) * 64, q_sl],
                            start=True, stop=True,
                            tile_position=(hl * 64, 0))
                    e = epool.tile([128, 1024], BF, tag="e",
                                   name=f"e_{b}_{qt}_{kc}")
                    nc.scalar.activation(
                        e[:], sT[:], mybir.ActivationFunctionType.Exp)
                    es.append(e)
                    # col-packed av: hl0 -> partitions 0:64, hl1 -> 64:128.
                    # has_written is per-element: each region's first matmul
                    # carries its own start flag.
                    for hl in range(2):
                        nc.tensor.matmul(
                            av[hl * 64:(hl + 1) * 64, :],
                            v_sb[:, 8 * b + kc, hl * 64:(hl + 1) * 64],
                            e[:, hl * 512:(hl + 1) * 512],
                            start=(kc == 0), stop=(kc == 7),
                            skip_group_check=True)
                    if kc % 2 == 1:
                        # 4 concurrent 1-col denominator matmuls (4 col
                        # groups x 4 distinct moving streams)
                        pair = kc // 2
                        for j, (ee, hl) in enumerate(
                                [(es[kc - 1], 0), (es[kc], 0),
                                 (es[kc - 1], 1), (es[kc], 1)]):
                            nc.tensor.matmul(
                                den[32 * j:32 * (j + 1), :], ones1[:],
                                ee[:, hl * 512:(hl + 1) * 512],
                                start=(pair == 0),
                                stop=(pair == 3),
                                skip_group_check=True,
                                tile_position=(0, 32 * j))
                return av, den

            def norm_block(b, qt, av, den):
                # den partials {0,32}=hl0 {64,96}=hl1 -> fp16 staging
                nc.vector.tensor_copy(out=den_sb[0:97, :], in_=den[0:97, :])
                # one PE matmul: pair-sum + broadcast across partitions
                bc = ps_den.tile([128, 512], F32, tag="den",
                                 name=f"bc_{b}_{qt}")
                nc.tensor.matmul(bc[:], smat[:], den_sb[:],
                                 start=True, stop=True)
                rb = spool.tile([128, 512], F32, tag="rb", name=f"rb_{b}_{qt}")
                nc.vector.reciprocal(rb[:], bc[:])
                nc.vector.tensor_mul(
                    projin[:, b, qt * 512:(qt + 1) * 512], av[:], rb[:])

            def proj_wave(nts, pool, ptag, reuse):
                for nt in nts:
                    if reuse:
                        pps0 = pool.tile([128, 512], F32, tag=ptag,
                                         name=f"pps0_{nt}")
                        pps1 = pool.tile([128, 512], F32, tag=ptag,
                                         name=f"pps1_{nt}")
                        for b2 in range(B):
                            st = projin[:, b2, nt * 128:(nt + 1) * 128]
                            nc.tensor.matmul(pps0[:], st, wp[:, b2, 0:512],
                                             start=(b2 == 0), stop=(b2 == 3))
                            nc.tensor.matmul(pps1[:], st, wp[:, b2, 512:1024],
                                             start=(b2 == 0), stop=(b2 == 3))
                        pps = [pps0, pps1]
                    else:
                        pps = []
                        for dt in range(2):
                            p = pool.tile([128, 512], F32, tag=ptag,
                                          name=f"pps{dt}_{nt}")
                            for b2 in range(B):
                                nc.tensor.matmul(
                                    p[:], projin[:, b2, nt * 128:(nt + 1) * 128],
                                    wp[:, b2, dt * 512:(dt + 1) * 512],
                                    start=(b2 == 0), stop=(b2 == 3))
                            pps.append(p)
                    for dt in range(2):
                        ocp = opool.tile([128, 512], F16, tag="o",
                                         name=f"ocp_{nt}_{dt}")
                        nc.vector.tensor_copy(out=ocp[:], in_=pps[dt][:])
                        nc.sync.dma_start(
                            out=out_e[nt * 128:(nt + 1) * 128,
                                      dt * 512:(dt + 1) * 512],
                            in_=ocp[:])

            # schedule: qkv one batch ahead of attention to keep PE dense;
            # first half of proj (n<512 needs only qt=0 outputs) overlaps the
            # last attention block
            qkv_block(0)
            for b in range(B):
                if b + 1 < B:
                    qkv_block(b + 1)
                for qt in range(2):
                    av, den = attn_block(b, qt)
                    norm_block(b, qt, av, den)
                    if b == B - 1 and qt == 0:
                        proj_wave(range(0, 4), ps_qkv, "qkv", reuse=False)
            attn_stk.close()
            with tc.tile_pool(name="ps_proj", bufs=4, space="PSUM") as ps_proj:
                proj_wave(range(4, 8), ps_proj, "pp", reuse=True)

    nc.compile()
    return nc


def _prep_core(i, xT, w_qkv, w_proj):
    """Per-core input shards (host-side layout absorption)."""
    h0 = 2 * i
    rows = np.concatenate([np.arange(h0 * HD, (h0 + 1) * HD),
                           np.arange((h0 + 1) * HD, (h0 + 2) * HD)])
    w_qk = np.concatenate([w_qkv[rows] * 0.125, w_qkv[C + rows]], axis=0).T
    w_v = w_qkv[2 * C + rows].T
    hh = np.array([h0, h0 + 1])
    cg = ((hh % 4)[None, :, None] * 256
          + np.arange(B)[:, None, None] * 64
          + np.arange(HD)[None, None, :])          # [b2, hl, d]
    w_p = w_proj[:, cg.reshape(-1)].T              # [512, 1024]
    return {
        "xT": xT,
        "w_qk": np.ascontiguousarray(w_qk, dtype=bf16),
        "w_v": np.ascontiguousarray(w_v, dtype=bf16),
        "w_p": np.ascontiguousarray(w_p, dtype=bf16),
    }


def _run(inputs, trace=False, **kw):
    x = np.asarray(inputs["x"], dtype=np.float32)
    w_qkv = np.asarray(inputs["w_qkv"], dtype=np.float32)
    w_proj = np.asarray(inputs["w_proj"], dtype=np.float32)
    b_proj = np.asarray(inputs["b_proj"], dtype=np.float32)

    if "nc" not in _NC_CACHE:
        _NC_CACHE["nc"] = build_nc()
    nc = _NC_CACHE["nc"]

    xT = np.ascontiguousarray(
        x.transpose(2, 1, 0).reshape(C, NT), dtype=bf16)
    in_maps = [_prep_core(i, xT, w_qkv, w_proj) for i in range(NCORES)]
    res = run_bass_kernel_spmd(nc, in_maps, core_ids=list(range(NCORES)),
                               trace=trace, **kw)
    out = np.empty((N, B, C), np.float32)
    for j in range(4):
        out[:, j, :] = (res.results[2 * j]["out"].astype(np.float32)
                        + res.results[2 * j + 1]["out"].astype(np.float32)
                        + b_proj)
    return out, res


def kernel(**inputs) -> np.ndarray:
    out, _ = _run(inputs, trace=False)
    return out


# revision 10
# speedup vs baseline: 1.0631x; 1.0631x over previous
"""Trainium2 8-core kernel for nn_Attention_88948772700322.

Reference computes (N=1024, B=4, C=1024, H=16, hd=64):
    qkv = x @ w_qkv.T                      [N,B,3C]
    q,k,v per (b,h); attn = softmax(q k^T / 8) v
    out = (attn.transpose(2,1,0,3)).reshape(N,B,C) @ w_proj.T + b_proj
The reshape interleaves H and B: proj-input channel c of output-batch bn is
attention head h = 4*bn + c//256, original batch b2 = (c%256)//64, dim d = c%64.

Sharding: tensor-parallel over heads — core i owns heads {2i, 2i+1}, all
batches/tokens (6.44 GFLOP/core, perfectly balanced).  Each core computes a
partial projection over its 512 proj-input channels for output batch bn=i//2;
host sums core pairs (the "all-reduce after proj" realized in unshard).

Host-side prep absorbs every layout nuisance:
  - xT [C, B*N] bf16, tokens batch-major  -> qkv needs no on-chip transpose
  - w_qk [C, 256] (cols q_h0,q_h1,k_h0,k_h1), q pre-scaled by 1/8
  - w_v  [C, 128] (cols v_h0,v_h1)
  - w_p  [512, 1024] = w_proj columns permuted to (b2, h_local, d) row order

On-chip per core (v2):
  - warm-up matmuls on memset data during the initial DMA wait (HAM ramp)
  - qk^T via PE (d-major), v via PE (token-major)
  - scores transposed (keys on partitions), exp without max-subtraction
  - av col-packed: both heads' 64-dim outputs in one PSUM bank via column
    tiling; softmax denominators via 4-up col-tiled ones-matmuls (4
    concurrent streams per kc pair)
  - normalization with zero DMAs: denominator pair-sum + partition-broadcast
    by one PE matmul against a constant selection matrix, then DVE
    reciprocal + multiply
  - partial proj n-major with stationary reuse; fp16 output (host upcasts)
"""

import numpy as np
import ml_dtypes

import concourse.bass as bass
import concourse.mybir as mybir
from concourse import bacc
from concourse.tile import TileContext
from concourse.bass_utils import run_bass_kernel_spmd


N, B, C, H, HD = 1024, 4, 1024, 16, 64
NT = B * N          # 4096 tokens
NCORES = 8
BF = mybir.dt.bfloat16
F16 = mybir.dt.float16
F32 = mybir.dt.float32
bf16 = ml_dtypes.bfloat16

_NC_CACHE = {}


def build_nc():
    nc = bacc.Bacc()
    xT_e = nc.declare_dram_parameter("xT", [C, NT], BF, isOutput=False)
    wqk_e = nc.declare_dram_parameter("w_qk", [C, 256], BF, isOutput=False)
    wv_e = nc.declare_dram_parameter("w_v", [C, 128], BF, isOutput=False)
    wp_e = nc.declare_dram_parameter("w_p", [512, C], BF, isOutput=False)
    out_e = nc.declare_dram_parameter("out", [N, C], F16, isOutput=True)

    xT_ap = xT_e[:].rearrange("(co p) t -> p co t", p=128)    # [128, 8, 4096]
    wqk_ap = wqk_e[:].rearrange("(co p) m -> p co m", p=128)  # [128, 8, 256]
    wv_ap = wv_e[:].rearrange("(co p) m -> p co m", p=128)    # [128, 8, 128]
    wp_ap = wp_e[:].rearrange("(b2 p) d -> p b2 d", p=128)    # [128, 4, 1024]

    from contextlib import ExitStack
    with TileContext(nc) as tc:
        with ExitStack() as stk:
            cpool = stk.enter_context(tc.tile_pool(name="const", bufs=1))
            epool = stk.enter_context(tc.tile_pool(name="exp", bufs=6))
            spool = stk.enter_context(tc.tile_pool(name="small", bufs=4))
            opool = stk.enter_context(tc.tile_pool(name="outcp", bufs=6))
            attn_stk = ExitStack()
            # PSUM budget (8 banks): qkv 1, sT 2x2, av 2, den/bcast 1
            ps_qkv = attn_stk.enter_context(
                tc.tile_pool(name="ps_qkv", bufs=2, space="PSUM"))
            ps_sT = attn_stk.enter_context(
                tc.tile_pool(name="ps_sT", bufs=3, space="PSUM"))
            ps_av = attn_stk.enter_context(
                tc.tile_pool(name="ps_av", bufs=2, space="PSUM"))
            ps_den = attn_stk.enter_context(
                tc.tile_pool(name="ps_den", bufs=1, space="PSUM"))
            # ---- persistent SBUF tensors -------------------------------
            xc = [[cpool.tile([128, N], BF, name=f"xc_{b}_{kc}")
                   for kc in range(8)] for b in range(B)]
            wqk = cpool.tile([128, 8, 256], BF)
            wv = cpool.tile([128, 8, 128], BF)
            wp = cpool.tile([128, 4, C], BF)
            q_sb = cpool.tile([128, NT], BF)       # [ (h0|h1) d, token ]
            k_sb = cpool.tile([128, NT], BF)
            v_sb = cpool.tile([128, 32, 128], BF)  # [t_in, t_chunk, (hl d)]
            projin = cpool.tile([128, B, N], BF)   # [(hl,d), b2, n]
            ones1 = cpool.tile([128, 32], BF)      # denominator stationary
            smat = cpool.tile([128, 128], F16)     # den pair-sum + broadcast
            den_sb = cpool.tile([128, 512], F16)   # den staging (bcast rhs)
            junk = cpool.tile([128, 512], BF)      # PE warm-up operand

            # constants first: warm-up matmuls depend only on these memsets
            nc.vector.memset(junk[:], 0.5)
            nc.vector.memset(ones1[:], 1.0)
            nc.vector.memset(den_sb[:], 1.0)
            nc.vector.memset(smat[:], 0.0)
            # bcast row p of output <- den partials: hl0 from psum partitions
            # {0,32}, hl1 from {64,96}
            nc.vector.memset(smat[0:1, 0:64], 1.0)
            nc.vector.memset(smat[32:33, 0:64], 1.0)
            nc.vector.memset(smat[64:65, 64:128], 1.0)
            nc.vector.memset(smat[96:97, 64:128], 1.0)

            # ---- HAM warm-up: keep PE streaming while input DMAs land --
            jp = ps_qkv.tile([128, 512], F32, tag="qkv", name="jp")
            for i in range(12):
                nc.tensor.matmul(jp[:], junk[:, 0:128], junk[:],
                                 start=True, stop=True)

            # ---- input DMAs -------------------------------------------
            # weights on the Scalar HWDGE queue, x on Sync — the two issue
            # streams run in parallel so x transfers start ~2us earlier.
            nc.scalar.dma_start(out=wqk[:], in_=wqk_ap)
            nc.scalar.dma_start(out=wv[:], in_=wv_ap)
            nc.scalar.dma_start(out=wp[:], in_=wp_ap)
            for b in range(B):
                for kc in range(8):
                    if b == 0 and kc < 2:  # halves: first matmul starts early
                        for hh in range(2):
                            sl = slice(hh * 512, (hh + 1) * 512)
                            nc.sync.dma_start(out=xc[b][kc][:, sl],
                                              in_=xT_ap[:, kc, b * N:(b + 1) * N][:, sl])
                    else:
                        nc.sync.dma_start(out=xc[b][kc][:],
                                          in_=xT_ap[:, kc, b * N:(b + 1) * N])

            def qkv_block(b):
                # q,k d-major: stationary = wqk column block, moving = x
                for tc_i in (2 * b, 2 * b + 1):
                    j = tc_i - 2 * b
                    for which in range(2):  # 0=q, 1=k
                        ps = ps_qkv.tile([128, 512], F32, tag="qkv",
                                         name=f"qk_{b}_{tc_i}_{which}")
                        for kc in range(8):
                            nc.tensor.matmul(
                                ps[:], wqk[:, kc, which * 128:(which + 1) * 128],
                                xc[b][kc][:, j * 512:(j + 1) * 512],
                                start=(kc == 0), stop=(kc == 7))
                        dst = q_sb if which == 0 else k_sb
                        nc.vector.tensor_copy(
                            out=dst[:, tc_i * 512:(tc_i + 1) * 512], in_=ps[:])
                # v token-major: stationary = x token block, moving = wv
                for tt in range(8 * b, 8 * b + 8):
                    vps = ps_qkv.tile([128, 128], F32, tag="qkv",
                                      name=f"vps_{tt}")
                    for kc in range(8):
                        nc.tensor.matmul(vps[:],
                                         xc[b][kc][:, (tt - 8 * b) * 128:
                                                    (tt - 8 * b + 1) * 128],
                                         wv[:, kc, :],
                                         start=(kc == 0), stop=(kc == 7))
                    nc.vector.tensor_copy(out=v_sb[:, tt, :], in_=vps[:])

            def attn_block(b, qt):
                q_sl = slice(b * N + qt * 512, b * N + (qt + 1) * 512)
                av = ps_av.tile([128, 512], F32, tag="av", name=f"av_{b}_{qt}")
                den = ps_den.tile([128, 512], F32, tag="den",
                                  name=f"den_{b}_{qt}")
                es = []
                for kc in range(8):
                    k_sl = slice(b * N + kc * 128, b * N + (kc + 1) * 128)
                    ee = []
                    for hl in range(2):
                        sT = ps_sT.tile([128, 512], F32, tag="sT",
                                        name=f"sT_{b}_{qt}_{kc}_{hl}")
                        nc.tensor.matmul(
                            sT[:],
                            k_sb[hl * 64:(hl + 1) * 64, k_sl],
                            q_sb[hl * 64:(hl + 1) * 64, q_sl],
                            start=True, stop=True,
                            tile_position=(hl * 64, 0))
                        e = epool.tile([128, 512], BF, tag="e",
                                       name=f"e_{b}_{qt}_{kc}_{hl}")
                        nc.scalar.activation(
                            e[:], sT[:], mybir.ActivationFunctionType.Exp)
                        ee.append(e)
                    es.append(ee)
                    # col-packed av: hl0 -> partitions 0:64, hl1 -> 64:128.
                    # has_written is per-element: each region's first matmul
                    # carries its own start flag.
                    for hl in range(2):
                        nc.tensor.matmul(
                            av[hl * 64:(hl + 1) * 64, :],
                            v_sb[:, 8 * b + kc, hl * 64:(hl + 1) * 64],
                            ee[hl][:],
                            start=(kc == 0), stop=(kc == 7),
                            skip_group_check=True)
                    if kc % 2 == 1:
                        # 4 concurrent denominator matmuls (4 col groups x 4
                        # distinct moving streams)
                        pair = kc // 2
                        for j, src in enumerate(
                                [es[kc - 1][0], es[kc][0],
                                 es[kc - 1][1], es[kc][1]]):
                            nc.tensor.matmul(
                                den[32 * j:32 * (j + 1), :], ones1[:],
                                src[:],
                                start=(pair == 0),
                                stop=(pair == 3),
                                skip_group_check=True,
                                tile_position=(0, 32 * j))
                return av, den

            def norm_block(b, qt, av, den):
                # den partials {0,32}=hl0 {64,96}=hl1 -> fp16 staging
                nc.vector.tensor_copy(out=den_sb[0:97, :], in_=den[0:97, :])
                # one PE matmul: pair-sum + broadcast across partitions
                bc = ps_sT.tile([128, 512], F32, tag="sT",
                                name=f"bc_{b}_{qt}")
                nc.tensor.matmul(bc[:], smat[:], den_sb[:],
                                 start=True, stop=True)
                rb = spool.tile([128, 512], F32, tag="rb", name=f"rb_{b}_{qt}")
                nc.vector.reciprocal_approx_fast(out=rb[:], in_=bc[:])
                nc.vector.tensor_mul(
                    projin[:, b, qt * 512:(qt + 1) * 512], av[:], rb[:])

            def proj_wave(nts, pool, ptag, reuse):
                for nt in nts:
                    if reuse:
                        pps0 = pool.tile([128, 512], F32, tag=ptag,
                                         name=f"pps0_{nt}")
                        pps1 = pool.tile([128, 512], F32, tag=ptag,
                                         name=f"pps1_{nt}")
                        for b2 in range(B):
                            st = projin[:, b2, nt * 128:(nt + 1) * 128]
                            nc.tensor.matmul(pps0[:], st, wp[:, b2, 0:512],
                                             start=(b2 == 0), stop=(b2 == 3))
                            nc.tensor.matmul(pps1[:], st, wp[:, b2, 512:1024],
                                             start=(b2 == 0), stop=(b2 == 3))
                        pps = [pps0, pps1]
                    else:
                        pps = []
                        for dt in range(2):
                            p = pool.tile([128, 512], F32, tag=ptag,
                                          name=f"pps{dt}_{nt}")
                            for b2 in range(B):
                                nc.tensor.matmul(
                                    p[:], projin[:, b2, nt * 128:(nt + 1) * 128],
                                    wp[:, b2, dt * 512:(dt + 1) * 512],
                                    start=(b2 == 0), stop=(b2 == 3))
                            pps.append(p)
                    for dt in range(2):
                        ocp = opool.tile([128, 512], F16, tag="o",
                                         name=f"ocp_{nt}_{dt}")
                        nc.vector.tensor_copy(out=ocp[:], in_=pps[dt][:])
                        nc.sync.dma_start(
                            out=out_e[nt * 128:(nt + 1) * 128,
                                      dt * 512:(dt + 1) * 512],
                            in_=ocp[:])

            # schedule: qkv one batch ahead of attention to keep PE dense;
            # first half of proj (n<512 needs only qt=0 outputs) overlaps the
            # last attention block
            qkv_block(0)
            for b in range(B):
                if b + 1 < B:
                    qkv_block(b + 1)
                for qt in range(2):
                    av, den = attn_block(b, qt)
                    norm_block(b, qt, av, den)
                    if b == B - 1 and qt == 0:
                        proj_wave(range(0, 4), ps_qkv, "qkv", reuse=False)
            attn_stk.close()
            with tc.tile_pool(name="ps_proj", bufs=4, space="PSUM") as ps_proj:
                proj_wave(range(4, 8), ps_proj, "pp", reuse=True)

    nc.compile()
    return nc


def _prep_core(i, xT, w_qkv, w_proj):
    """Per-core input shards (host-side layout absorption)."""
    h0 = 2 * i
    rows = np.concatenate([np.arange(h0 * HD, (h0 + 1) * HD),
                           np.arange((h0 + 1) * HD, (h0 + 2) * HD)])
    w_qk = np.concatenate([w_qkv[rows] * 0.125, w_qkv[C + rows]], axis=0).T
    w_v = w_qkv[2 * C + rows].T
    hh = np.array([h0, h0 + 1])
    cg = ((hh % 4)[None, :, None] * 256
          + np.arange(B)[:, None, None] * 64
          + np.arange(HD)[None, None, :])          # [b2, hl, d]
    w_p = w_proj[:, cg.reshape(-1)].T              # [512, 1024]
    return {
        "xT": xT,
        "w_qk": np.ascontiguousarray(w_qk, dtype=bf16),
        "w_v": np.ascontiguousarray(w_v, dtype=bf16),
        "w_p": np.ascontiguousarray(w_p, dtype=bf16),
    }


def _run(inputs, trace=False, **kw):
    x = np.asarray(inputs["x"], dtype=np.float32)
    w_qkv = np.asarray(inputs["w_qkv"], dtype=np.float32)
    w_proj = np.asarray(inputs["w_proj"], dtype=np.float32)
    b_proj = np.asarray(inputs["b_proj"], dtype=np.float32)

    if "nc" not in _NC_CACHE:
        _NC_CACHE["nc"] = build_nc()
    nc = _NC_CACHE["nc"]

    xT = np.ascontiguousarray(
        x.transpose(2, 1, 0).reshape(C, NT), dtype=bf16)
    in_maps = [_prep_core(i, xT, w_qkv, w_proj) for i in range(NCORES)]
    res = run_bass_kernel_spmd(nc, in_maps, core_ids=list(range(NCORES)),
                               trace=trace, **kw)
    out = np.empty((N, B, C), np.float32)
    for j in range(4):
        out[:, j, :] = (res.results[2 * j]["out"].astype(np.float32)
                        + res.results[2 * j + 1]["out"].astype(np.float32)
                        + b_proj)
    return out, res


def kernel(**inputs) -> np.ndarray:
    out, _ = _run(inputs, trace=False)
    return out


# revision 14
# speedup vs baseline: 1.2298x; 1.1568x over previous
"""Trainium2 8-core kernel for nn_Attention_88948772700322.

Reference computes (N=1024, B=4, C=1024, H=16, hd=64):
    qkv = x @ w_qkv.T                      [N,B,3C]
    q,k,v per (b,h); attn = softmax(q k^T / 8) v
    out = (attn.transpose(2,1,0,3)).reshape(N,B,C) @ w_proj.T + b_proj
The reshape interleaves H and B: proj-input channel c of output-batch bn is
attention head h = 4*bn + c//256, original batch b2 = (c%256)//64, dim d = c%64.

Sharding: tensor-parallel over heads — core i owns heads {2i, 2i+1}, all
batches/tokens (6.44 GFLOP/core, perfectly balanced).  Each core computes a
partial projection over its 512 proj-input channels for output batch bn=i//2;
host sums core pairs (the "all-reduce after proj" realized in unshard).

Host-side prep absorbs every layout nuisance:
  - xT [C, B*N] bf16, tokens batch-major  -> qkv needs no on-chip transpose
  - w_qk [C, 256] (cols q_h0,q_h1,k_h0,k_h1), q pre-scaled by 1/8
  - w_v  [C, 128] (cols v_h0,v_h1)
  - w_p  [512, 1024] = w_proj columns permuted to (b2, h_local, d) row order

On-chip per core (v2):
  - warm-up matmuls on memset data during the initial DMA wait (HAM ramp)
  - qk^T via PE (d-major), v via PE (token-major)
  - scores transposed (keys on partitions), exp without max-subtraction
  - av col-packed: both heads' 64-dim outputs in one PSUM bank via column
    tiling; softmax denominators via 4-up col-tiled ones-matmuls (4
    concurrent streams per kc pair)
  - normalization with zero DMAs: denominator pair-sum + partition-broadcast
    by one PE matmul against a constant selection matrix, then DVE
    reciprocal + multiply
  - partial proj n-major with stationary reuse; fp16 output (host upcasts)
"""

import numpy as np
import ml_dtypes

import concourse.bass as bass
import concourse.mybir as mybir
from concourse import bacc
from concourse.tile import TileContext
from concourse.bass_utils import run_bass_kernel_spmd


N, B, C, H, HD = 1024, 4, 1024, 16, 64
NT = B * N          # 4096 tokens
NCORES = 8
BF = mybir.dt.bfloat16
F16 = mybir.dt.float16
F32 = mybir.dt.float32
bf16 = ml_dtypes.bfloat16

_NC_CACHE = {}


def build_nc():
    nc = bacc.Bacc()
    xT_e = nc.declare_dram_parameter("xT", [C, NT], BF, isOutput=False)
    wqk_e = nc.declare_dram_parameter("w_qk", [C, 256], BF, isOutput=False)
    wv_e = nc.declare_dram_parameter("w_v", [C, 128], BF, isOutput=False)
    wp_e = nc.declare_dram_parameter("w_p", [512, C], BF, isOutput=False)
    out_e = nc.declare_dram_parameter("out", [N, C], F16, isOutput=True)

    xT_ap = xT_e[:].rearrange("(co p) t -> p co t", p=128)    # [128, 8, 4096]
    wqk_ap = wqk_e[:].rearrange("(co p) m -> p co m", p=128)  # [128, 8, 256]
    wv_ap = wv_e[:].rearrange("(co p) m -> p co m", p=128)    # [128, 8, 128]
    wp_ap = wp_e[:].rearrange("(b2 p) d -> p b2 d", p=128)    # [128, 4, 1024]

    from contextlib import ExitStack
    with TileContext(nc) as tc:
        with ExitStack() as stk:
            cpool = stk.enter_context(tc.tile_pool(name="const", bufs=1))
            epool = stk.enter_context(tc.tile_pool(name="exp", bufs=6))
            spool = stk.enter_context(tc.tile_pool(name="small", bufs=4))
            opool = stk.enter_context(tc.tile_pool(name="outcp", bufs=6))
            attn_stk = ExitStack()
            # PSUM budget (8 banks): qkv 1, sT 2x2, av 2, den/bcast 1
            ps_qkv = attn_stk.enter_context(
                tc.tile_pool(name="ps_qkv", bufs=2, space="PSUM"))
            ps_sT = attn_stk.enter_context(
                tc.tile_pool(name="ps_sT", bufs=2, space="PSUM"))
            ps_av = attn_stk.enter_context(
                tc.tile_pool(name="ps_av", bufs=2, space="PSUM"))
            # ---- persistent SBUF tensors -------------------------------
            xc = [[cpool.tile([128, N], BF, name=f"xc_{b}_{kc}")
                   for kc in range(8)] for b in range(B)]
            wqk = cpool.tile([128, 8, 256], BF)
            wv = cpool.tile([128, 8, 128], BF)
            wp = cpool.tile([128, 4, C], BF)
            q_sb = cpool.tile([128, NT], BF)       # [ (h0|h1) d, token ]
            k_sb = cpool.tile([128, NT], BF)
            v_sb = cpool.tile([128, 32, 130], BF)  # [t_in, tt, (d,1,d,1)]
            projin = cpool.tile([128, B, N], BF)   # [(hl,d), b2, n]
            onesP = cpool.tile([128, 128], F16)    # bcast stationary (row 64)
            den_sb = cpool.tile([128, 1024], F16)  # den staging on partition 64
            av_all = cpool.tile([128, 512], F32)   # both heads' av, aligned
            junk = cpool.tile([128, 512], BF)      # PE warm-up operand

            # constants first: warm-up matmuls depend only on these memsets
            nc.vector.memset(junk[:], 0.5)
            nc.vector.memset(onesP[:], 0.0)
            nc.vector.memset(onesP[64:65, :], 1.0)
            nc.vector.memset(den_sb[:], 1.0)
            nc.vector.memset(v_sb[:, :, 64:65], 1.0)
            nc.vector.memset(v_sb[:, :, 129:130], 1.0)

            # ---- HAM warm-up: keep PE streaming while input DMAs land --
            jp = ps_qkv.tile([128, 512], F32, tag="qkv", name="jp")
            for i in range(12):
                nc.tensor.matmul(jp[:], junk[:, 0:128], junk[:],
                                 start=True, stop=True)

            # ---- input DMAs -------------------------------------------
            # weights on the Scalar HWDGE queue, x on Sync — the two issue
            # streams run in parallel so x transfers start ~2us earlier.
            nc.scalar.dma_start(out=wqk[:], in_=wqk_ap)
            nc.scalar.dma_start(out=wv[:], in_=wv_ap)
            nc.scalar.dma_start(out=wp[:], in_=wp_ap)
            for b in range(B):
                for kc in range(8):
                    if b == 0 and kc < 2:  # halves: first matmul starts early
                        for hh in range(2):
                            sl = slice(hh * 512, (hh + 1) * 512)
                            nc.sync.dma_start(out=xc[b][kc][:, sl],
                                              in_=xT_ap[:, kc, b * N:(b + 1) * N][:, sl])
                    else:
                        nc.sync.dma_start(out=xc[b][kc][:],
                                          in_=xT_ap[:, kc, b * N:(b + 1) * N])

            def qkv_block(b):
                # q,k d-major: stationary = wqk column block, moving = x
                for tc_i in (2 * b, 2 * b + 1):
                    j = tc_i - 2 * b
                    for which in range(2):  # 0=q, 1=k
                        ps = ps_qkv.tile([128, 512], F32, tag="qkv",
                                         name=f"qk_{b}_{tc_i}_{which}")
                        for kc in range(8):
                            nc.tensor.matmul(
                                ps[:], wqk[:, kc, which * 128:(which + 1) * 128],
                                xc[b][kc][:, j * 512:(j + 1) * 512],
                                start=(kc == 0), stop=(kc == 7))
                        dst = q_sb if which == 0 else k_sb
                        nc.vector.tensor_copy(
                            out=dst[:, tc_i * 512:(tc_i + 1) * 512], in_=ps[:])
                # v token-major: stationary = x token block, moving = wv
                for tt in range(8 * b, 8 * b + 8):
                    vps = ps_qkv.tile([128, 128], F32, tag="qkv",
                                      name=f"vps_{tt}")
                    for kc in range(8):
                        nc.tensor.matmul(vps[:],
                                         xc[b][kc][:, (tt - 8 * b) * 128:
                                                    (tt - 8 * b + 1) * 128],
                                         wv[:, kc, :],
                                         start=(kc == 0), stop=(kc == 7))
                    nc.vector.tensor_copy(out=v_sb[:, tt, 0:64],
                                          in_=vps[:, 0:64])
                    nc.vector.tensor_copy(out=v_sb[:, tt, 65:129],
                                          in_=vps[:, 64:128])

            def attn_block(b, qt):
                q_sl = slice(b * N + qt * 512, b * N + (qt + 1) * 512)
                av0 = ps_av.tile([65, 512], F32, tag="av", name=f"av0_{b}_{qt}")
                av1 = ps_av.tile([65, 512], F32, tag="av", name=f"av1_{b}_{qt}")
                avs = [av0, av1]
                for kc in range(8):
                    k_sl = slice(b * N + kc * 128, b * N + (kc + 1) * 128)
                    sT = ps_sT.tile([128, 1024], F32, tag="sT",
                                    name=f"sT_{b}_{qt}_{kc}")
                    for hl in range(2):
                        nc.tensor.matmul(
                            sT[:, hl * 512:(hl + 1) * 512],
                            k_sb[hl * 64:(hl + 1) * 64, k_sl],
                            q_sb[hl * 64:(hl + 1) * 64, q_sl],
                            start=True, stop=True,
                            tile_position=(hl * 64, 0))
                    e = epool.tile([128, 1024], BF, tag="e",
                                   name=f"e_{b}_{qt}_{kc}")
                    nc.scalar.activation(
                        e[:], sT[:], mybir.ActivationFunctionType.Exp)
                    # fused av: 64 dims + ones column (softmax denominator)
                    for hl in range(2):
                        nc.tensor.matmul(
                            avs[hl][:],
                            v_sb[:, 8 * b + kc, hl * 65:(hl + 1) * 65],
                            e[:, hl * 512:(hl + 1) * 512],
                            start=(kc == 0), stop=(kc == 7))
                return av0, av1

            def norm_block(b, qt, av0, av1):
                # denominators (row 64 of each av bank) -> fp16 staging on
                # partition 64; dims -> av_all aligned [hl0 | hl1]
                nc.vector.tensor_copy(out=den_sb[64:65, 0:512],
                                      in_=av0[64:65, :])
                nc.vector.tensor_copy(out=den_sb[64:65, 512:1024],
                                      in_=av1[64:65, :])
                nc.vector.tensor_copy(out=av_all[0:64, :], in_=av0[0:64, :])
                nc.vector.tensor_copy(out=av_all[64:128, :], in_=av1[0:64, :])
                # two 1-row PE matmuls broadcast the denominators across
                # partitions: bc[0:64] = den_h0, bc[64:128] = den_h1
                bc = ps_sT.tile([128, 512], F32, tag="sT",
                                name=f"bc_{b}_{qt}")
                nc.tensor.matmul(bc[0:64, :], onesP[64:65, 0:64],
                                 den_sb[64:65, 0:512], start=True, stop=True,
                                 tile_position=(64, 0))
                nc.tensor.matmul(bc[64:128, :], onesP[64:65, 64:128],
                                 den_sb[64:65, 512:1024], start=True,
                                 stop=True, tile_position=(64, 64),
                                 skip_group_check=True)
                rb = spool.tile([128, 512], F32, tag="rb", name=f"rb_{b}_{qt}")
                nc.vector.reciprocal_approx_fast(out=rb[:], in_=bc[:])
                nc.vector.tensor_mul(
                    projin[:, b, qt * 512:(qt + 1) * 512], av_all[:], rb[:])

            def proj_wave(nts, pool, ptag, reuse):
                for nt in nts:
                    if reuse:
                        pps0 = pool.tile([128, 512], F32, tag=ptag,
                                         name=f"pps0_{nt}")
                        pps1 = pool.tile([128, 512], F32, tag=ptag,
                                         name=f"pps1_{nt}")
                        for b2 in range(B):
                            st = projin[:, b2, nt * 128:(nt + 1) * 128]
                            nc.tensor.matmul(pps0[:], st, wp[:, b2, 0:512],
                                             start=(b2 == 0), stop=(b2 == 3))
                            nc.tensor.matmul(pps1[:], st, wp[:, b2, 512:1024],
                                             start=(b2 == 0), stop=(b2 == 3))
                        pps = [pps0, pps1]
                    else:
                        pps = []
                        for dt in range(2):
                            p = pool.tile([128, 512], F32, tag=ptag,
                                          name=f"pps{dt}_{nt}")
                            for b2 in range(B):
                                nc.tensor.matmul(
                                    p[:], projin[:, b2, nt * 128:(nt + 1) * 128],
                                    wp[:, b2, dt * 512:(dt + 1) * 512],
                                    start=(b2 == 0), stop=(b2 == 3))
                            pps.append(p)
                    for dt in range(2):
                        ocp = opool.tile([128, 512], F16, tag="o",
                                         name=f"ocp_{nt}_{dt}")
                        nc.vector.tensor_copy(out=ocp[:], in_=pps[dt][:])
                        nc.sync.dma_start(
                            out=out_e[nt * 128:(nt + 1) * 128,
                                      dt * 512:(dt + 1) * 512],
                            in_=ocp[:])

            # schedule: qkv one batch ahead of attention to keep PE dense;
            # first half of proj (n<512 needs only qt=0 outputs) overlaps the
            # last attention block
            qkv_block(0)
            for b in range(B):
                if b + 1 < B:
                    qkv_block(b + 1)
                for qt in range(2):
                    av0, av1 = attn_block(b, qt)
                    norm_block(b, qt, av0, av1)
                    if b == B - 1 and qt == 0:
                        proj_wave(range(0, 4), ps_qkv, "qkv", reuse=False)
            attn_stk.close()
            with tc.tile_pool(name="ps_proj", bufs=4, space="PSUM") as ps_proj:
                proj_wave(range(4, 8), ps_proj, "pp", reuse=True)

    nc.compile()
    return nc


def _prep_core(i, xT, w_qkv, w_proj):
    """Per-core input shards (host-side layout absorption)."""
    h0 = 2 * i
    rows = np.concatenate([np.arange(h0 * HD, (h0 + 1) * HD),
                           np.arange((h0 + 1) * HD, (h0 + 2) * HD)])
    w_qk = np.concatenate([w_qkv[rows] * 0.125, w_qkv[C + rows]], axis=0).T
    w_v = w_qkv[2 * C + rows].T
    hh = np.array([h0, h0 + 1])
    cg = ((hh % 4)[None, :, None] * 256
          + np.arange(B)[:, None, None] * 64
          + np.arange(HD)[None, None, :])          # [b2, hl, d]
    w_p = w_proj[:, cg.reshape(-1)].T              # [512, 1024]
    return {
        "xT": xT,
        "w_qk": np.ascontiguousarray(w_qk, dtype=bf16),
        "w_v": np.ascontiguousarray(w_v, dtype=bf16),
        "w_p": np.ascontiguousarray(w_p, dtype=bf16),
    }


def _run(inputs, trace=False, **kw):
    x = np.asarray(inputs["x"], dtype=np.float32)
    w_qkv = np.asarray(inputs["w_qkv"], dtype=np.float32)
    w_proj = np.asarray(inputs["w_proj"], dtype=np.float32)
    b_proj = np.asarray(inputs["b_proj"], dtype=np.float32)

    if "nc" not in _NC_CACHE:
        _NC_CACHE["nc"] = build_nc()
    nc = _NC_CACHE["nc"]

    xT = np.ascontiguousarray(
        x.transpose(2, 1, 0).reshape(C, NT), dtype=bf16)
    in_maps = [_prep_core(i, xT, w_qkv, w_proj) for i in range(NCORES)]
    res = run_bass_kernel_spmd(nc, in_maps, core_ids=list(range(NCORES)),
                               trace=trace, **kw)
    out = np.empty((N, B, C), np.float32)
    for j in range(4):
        out[:, j, :] = (res.results[2 * j]["out"].astype(np.float32)
                        + res.results[2 * j + 1]["out"].astype(np.float32)
                        + b_proj)
    return out, res


def kernel(**inputs) -> np.ndarray:
    out, _ = _run(inputs, trace=False)
    return out


# revision 16
# speedup vs baseline: 1.2460x; 1.0132x over previous
"""Trainium2 8-core kernel for nn_Attention_88948772700322.

Reference computes (N=1024, B=4, C=1024, H=16, hd=64):
    qkv = x @ w_qkv.T                      [N,B,3C]
    q,k,v per (b,h); attn = softmax(q k^T / 8) v
    out = (attn.transpose(2,1,0,3)).reshape(N,B,C) @ w_proj.T + b_proj
The reshape interleaves H and B: proj-input channel c of output-batch bn is
attention head h = 4*bn + c//256, original batch b2 = (c%256)//64, dim d = c%64.

Sharding: tensor-parallel over heads — core i owns heads {2i, 2i+1}, all
batches/tokens (6.44 GFLOP/core, perfectly balanced).  Each core computes a
partial projection over its 512 proj-input channels for output batch bn=i//2;
host sums core pairs (the "all-reduce after proj" realized in unshard).

Host-side prep absorbs every layout nuisance:
  - xT [C, B*N] bf16, tokens batch-major  -> qkv needs no on-chip transpose
  - w_qk [C, 256] (cols q_h0,q_h1,k_h0,k_h1), q pre-scaled by 1/8
  - w_v  [C, 128] (cols v_h0,v_h1)
  - w_p  [512, 1024] = w_proj columns permuted to (b2, h_local, d) row order

On-chip per core (v2):
  - warm-up matmuls on memset data during the initial DMA wait (HAM ramp)
  - qk^T via PE (d-major), v via PE (token-major)
  - scores transposed (keys on partitions), exp without max-subtraction
  - av col-packed: both heads' 64-dim outputs in one PSUM bank via column
    tiling; softmax denominators via 4-up col-tiled ones-matmuls (4
    concurrent streams per kc pair)
  - normalization with zero DMAs: denominator pair-sum + partition-broadcast
    by one PE matmul against a constant selection matrix, then DVE
    reciprocal + multiply
  - partial proj n-major with stationary reuse; fp16 output (host upcasts)
"""

import numpy as np
import ml_dtypes

import concourse.bass as bass
import concourse.mybir as mybir
from concourse import bacc
from concourse.tile import TileContext
from concourse.bass_utils import run_bass_kernel_spmd


N, B, C, H, HD = 1024, 4, 1024, 16, 64
NT = B * N          # 4096 tokens
NCORES = 8
BF = mybir.dt.bfloat16
F16 = mybir.dt.float16
F32 = mybir.dt.float32
bf16 = ml_dtypes.bfloat16

_NC_CACHE = {}


def build_nc():
    nc = bacc.Bacc()
    xT_e = nc.declare_dram_parameter("xT", [C, NT], BF, isOutput=False)
    wqk_e = nc.declare_dram_parameter("w_qk", [C, 256], BF, isOutput=False)
    wv_e = nc.declare_dram_parameter("w_v", [C, 128], BF, isOutput=False)
    wp_e = nc.declare_dram_parameter("w_p", [512, C], BF, isOutput=False)
    out_e = nc.declare_dram_parameter("out", [N, C], F16, isOutput=True)

    xT_ap = xT_e[:].rearrange("(co p) t -> p co t", p=128)    # [128, 8, 4096]
    wqk_ap = wqk_e[:].rearrange("(co p) m -> p co m", p=128)  # [128, 8, 256]
    wv_ap = wv_e[:].rearrange("(co p) m -> p co m", p=128)    # [128, 8, 128]
    wp_ap = wp_e[:].rearrange("(b2 p) d -> p b2 d", p=128)    # [128, 4, 1024]

    from contextlib import ExitStack
    with TileContext(nc) as tc:
        with ExitStack() as stk:
            cpool = stk.enter_context(tc.tile_pool(name="const", bufs=1))
            epool = stk.enter_context(tc.tile_pool(name="exp", bufs=6))
            spool = stk.enter_context(tc.tile_pool(name="small", bufs=4))
            opool = stk.enter_context(tc.tile_pool(name="outcp", bufs=6))
            attn_stk = ExitStack()
            # PSUM budget (8 banks): qkv 1, sT 2x2, av 2, den/bcast 1
            ps_qkv = attn_stk.enter_context(
                tc.tile_pool(name="ps_qkv", bufs=2, space="PSUM"))
            ps_sT = attn_stk.enter_context(
                tc.tile_pool(name="ps_sT", bufs=2, space="PSUM"))
            ps_av = attn_stk.enter_context(
                tc.tile_pool(name="ps_av", bufs=2, space="PSUM"))
            # ---- persistent SBUF tensors -------------------------------
            xc = [[cpool.tile([128, N], BF, name=f"xc_{b}_{kc}")
                   for kc in range(8)] for b in range(B)]
            wqk = cpool.tile([128, 8, 256], BF)
            wv = cpool.tile([128, 8, 128], BF)
            wp = cpool.tile([128, 4, C], BF)
            q_sb = cpool.tile([128, NT], BF)       # [ (h0|h1) d, token ]
            k_sb = cpool.tile([128, NT], BF)
            v_sb = cpool.tile([128, 32, 130], BF)  # [t_in, tt, (d,1,d,1)]
            projin = cpool.tile([128, B, N], BF)   # [(hl,d), b2, n]
            onesP = cpool.tile([128, 128], F16)    # bcast stationary (row 64)
            den_sb = cpool.tile([128, 1024], F16)  # den staging on partition 64
            av_all = cpool.tile([128, 512], F32)   # both heads' av, aligned
            acc = cpool.tile([128, 8, 1024], F32)  # proj partial accumulator
            junk = cpool.tile([128, 512], BF)      # PE warm-up operand

            # constants first: warm-up matmuls depend only on these memsets
            nc.vector.memset(junk[:], 0.5)
            nc.vector.memset(onesP[:], 0.0)
            nc.vector.memset(onesP[64:65, :], 1.0)
            nc.vector.memset(den_sb[:], 1.0)
            nc.vector.memset(v_sb[:, :, 64:65], 1.0)
            nc.vector.memset(v_sb[:, :, 129:130], 1.0)

            # ---- HAM warm-up: keep PE streaming while input DMAs land --
            jp = ps_qkv.tile([128, 512], F32, tag="qkv", name="jp")
            for i in range(12):
                nc.tensor.matmul(jp[:], junk[:, 0:128], junk[:],
                                 start=True, stop=True)

            # ---- input DMAs -------------------------------------------
            # weights on the Scalar HWDGE queue, x on Sync — the two issue
            # streams run in parallel so x transfers start ~2us earlier.
            nc.scalar.dma_start(out=wqk[:], in_=wqk_ap)
            nc.scalar.dma_start(out=wv[:], in_=wv_ap)
            nc.scalar.dma_start(out=wp[:], in_=wp_ap)
            for b in range(B):
                for kc in range(8):
                    if b == 0 and kc < 2:  # halves: first matmul starts early
                        for hh in range(2):
                            sl = slice(hh * 512, (hh + 1) * 512)
                            nc.sync.dma_start(out=xc[b][kc][:, sl],
                                              in_=xT_ap[:, kc, b * N:(b + 1) * N][:, sl])
                    else:
                        nc.sync.dma_start(out=xc[b][kc][:],
                                          in_=xT_ap[:, kc, b * N:(b + 1) * N])

            def qkv_block(b):
                # q,k d-major: stationary = wqk column block, moving = x
                for tc_i in (2 * b, 2 * b + 1):
                    j = tc_i - 2 * b
                    for which in range(2):  # 0=q, 1=k
                        ps = ps_qkv.tile([128, 512], F32, tag="qkv",
                                         name=f"qk_{b}_{tc_i}_{which}")
                        for kc in range(8):
                            nc.tensor.matmul(
                                ps[:], wqk[:, kc, which * 128:(which + 1) * 128],
                                xc[b][kc][:, j * 512:(j + 1) * 512],
                                start=(kc == 0), stop=(kc == 7))
                        dst = q_sb if which == 0 else k_sb
                        nc.vector.tensor_copy(
                            out=dst[:, tc_i * 512:(tc_i + 1) * 512], in_=ps[:])
                # v token-major: stationary = x token block, moving = wv
                for tt in range(8 * b, 8 * b + 8):
                    vps = ps_qkv.tile([128, 128], F32, tag="qkv",
                                      name=f"vps_{tt}")
                    for kc in range(8):
                        nc.tensor.matmul(vps[:],
                                         xc[b][kc][:, (tt - 8 * b) * 128:
                                                    (tt - 8 * b + 1) * 128],
                                         wv[:, kc, :],
                                         start=(kc == 0), stop=(kc == 7))
                    nc.vector.tensor_copy(out=v_sb[:, tt, 0:64],
                                          in_=vps[:, 0:64])
                    nc.vector.tensor_copy(out=v_sb[:, tt, 65:129],
                                          in_=vps[:, 64:128])

            def attn_block(b, qt):
                q_sl = slice(b * N + qt * 512, b * N + (qt + 1) * 512)
                av0 = ps_av.tile([65, 512], F32, tag="av", name=f"av0_{b}_{qt}")
                av1 = ps_av.tile([65, 512], F32, tag="av", name=f"av1_{b}_{qt}")
                avs = [av0, av1]
                for kc in range(8):
                    k_sl = slice(b * N + kc * 128, b * N + (kc + 1) * 128)
                    sT = ps_sT.tile([128, 1024], F32, tag="sT",
                                    name=f"sT_{b}_{qt}_{kc}")
                    for hl in range(2):
                        nc.tensor.matmul(
                            sT[:, hl * 512:(hl + 1) * 512],
                            k_sb[hl * 64:(hl + 1) * 64, k_sl],
                            q_sb[hl * 64:(hl + 1) * 64, q_sl],
                            start=True, stop=True,
                            tile_position=(hl * 64, 0))
                    e = epool.tile([128, 1024], BF, tag="e",
                                   name=f"e_{b}_{qt}_{kc}")
                    nc.scalar.activation(
                        e[:], sT[:], mybir.ActivationFunctionType.Exp)
                    # fused av: 64 dims + ones column (softmax denominator)
                    for hl in range(2):
                        nc.tensor.matmul(
                            avs[hl][:],
                            v_sb[:, 8 * b + kc, hl * 65:(hl + 1) * 65],
                            e[:, hl * 512:(hl + 1) * 512],
                            start=(kc == 0), stop=(kc == 7))
                return av0, av1

            def norm_block(b, qt, av0, av1):
                # denominators (row 64 of each av bank) -> fp16 staging on
                # partition 64; dims -> av_all aligned [hl0 | hl1]
                nc.vector.tensor_copy(out=den_sb[64:65, 0:512],
                                      in_=av0[64:65, :])
                nc.vector.tensor_copy(out=den_sb[64:65, 512:1024],
                                      in_=av1[64:65, :])
                nc.vector.tensor_copy(out=av_all[0:64, :], in_=av0[0:64, :])
                nc.vector.tensor_copy(out=av_all[64:128, :], in_=av1[0:64, :])
                # two 1-row PE matmuls broadcast the denominators across
                # partitions: bc[0:64] = den_h0, bc[64:128] = den_h1
                bc = ps_sT.tile([128, 512], F32, tag="sT",
                                name=f"bc_{b}_{qt}")
                nc.tensor.matmul(bc[0:64, :], onesP[64:65, 0:64],
                                 den_sb[64:65, 0:512], start=True, stop=True,
                                 tile_position=(64, 0))
                nc.tensor.matmul(bc[64:128, :], onesP[64:65, 64:128],
                                 den_sb[64:65, 512:1024], start=True,
                                 stop=True, tile_position=(64, 64),
                                 skip_group_check=True)
                rb = spool.tile([128, 512], F32, tag="rb", name=f"rb_{b}_{qt}")
                nc.vector.reciprocal_approx_fast(out=rb[:], in_=bc[:])
                nc.vector.tensor_mul(
                    projin[:, b, qt * 512:(qt + 1) * 512], av_all[:], rb[:])

            def proj_pair(nts, b2lo):
                # partial projection over batches (b2lo, b2lo+1): first pair
                # seeds the SBUF accumulator, second pair fuses add + fp16
                # cast and streams the chunk out.
                for nt in nts:
                    for dt in range(2):
                        pps = ps_qkv.tile([128, 512], F32, tag="qkv",
                                          name=f"pps_{nt}_{dt}_{b2lo}")
                        for b2 in (b2lo, b2lo + 1):
                            nc.tensor.matmul(
                                pps[:], projin[:, b2, nt * 128:(nt + 1) * 128],
                                wp[:, b2, dt * 512:(dt + 1) * 512],
                                start=(b2 == b2lo), stop=(b2 == b2lo + 1))
                        a_sl = acc[:, nt, dt * 512:(dt + 1) * 512]
                        if b2lo == 0:
                            nc.vector.tensor_copy(out=a_sl, in_=pps[:])
                        else:
                            ocp = opool.tile([128, 512], F16, tag="o",
                                             name=f"ocp_{nt}_{dt}")
                            nc.vector.tensor_add(ocp[:], pps[:], a_sl)
                            nc.sync.dma_start(
                                out=out_e[nt * 128:(nt + 1) * 128,
                                          dt * 512:(dt + 1) * 512],
                                in_=ocp[:])

            # schedule: qkv one batch ahead of attention; proj partial-pair
            # waves are woven in as soon as their norms exist so the PE never
            # drains in the tail.
            qkv_block(0)
            for b in range(B):
                if b + 1 < B:
                    qkv_block(b + 1)
                for qt in range(2):
                    av0, av1 = attn_block(b, qt)
                    norm_block(b, qt, av0, av1)
                    if b == 2 and qt == 0:
                        proj_pair(range(0, 4), 0)   # norms (0,0),(1,0) ready
                    elif b == 2 and qt == 1:
                        proj_pair(range(4, 8), 0)   # norms (0,1),(1,1) ready
                    elif b == 3 and qt == 0:
                        proj_pair(range(0, 4), 2)   # norms (2,0),(3,0) ready
                    elif b == 3 and qt == 1:
                        proj_pair(range(4, 8), 2)
            attn_stk.close()

    nc.compile()
    return nc


def _prep_core(i, xT, w_qkv, w_proj):
    """Per-core input shards (host-side layout absorption)."""
    h0 = 2 * i
    rows = np.concatenate([np.arange(h0 * HD, (h0 + 1) * HD),
                           np.arange((h0 + 1) * HD, (h0 + 2) * HD)])
    w_qk = np.concatenate([w_qkv[rows] * 0.125, w_qkv[C + rows]], axis=0).T
    w_v = w_qkv[2 * C + rows].T
    hh = np.array([h0, h0 + 1])
    cg = ((hh % 4)[None, :, None] * 256
          + np.arange(B)[:, None, None] * 64
          + np.arange(HD)[None, None, :])          # [b2, hl, d]
    w_p = w_proj[:, cg.reshape(-1)].T              # [512, 1024]
    return {
        "xT": xT,
        "w_qk": np.ascontiguousarray(w_qk, dtype=bf16),
        "w_v": np.ascontiguousarray(w_v, dtype=bf16),
        "w_p": np.ascontiguousarray(w_p, dtype=bf16),
    }


def _run(inputs, trace=False, **kw):
    x = np.asarray(inputs["x"], dtype=np.float32)
    w_qkv = np.asarray(inputs["w_qkv"], dtype=np.float32)
    w_proj = np.asarray(inputs["w_proj"], dtype=np.float32)
    b_proj = np.asarray(inputs["b_proj"], dtype=np.float32)

    if "nc" not in _NC_CACHE:
        _NC_CACHE["nc"] = build_nc()
    nc = _NC_CACHE["nc"]

    xT = np.ascontiguousarray(
        x.transpose(2, 1, 0).reshape(C, NT), dtype=bf16)
    in_maps = [_prep_core(i, xT, w_qkv, w_proj) for i in range(NCORES)]
    res = run_bass_kernel_spmd(nc, in_maps, core_ids=list(range(NCORES)),
                               trace=trace, **kw)
    out = np.empty((N, B, C), np.float32)
    for j in range(4):
        out[:, j, :] = (res.results[2 * j]["out"].astype(np.float32)
                        + res.results[2 * j + 1]["out"].astype(np.float32)
                        + b_proj)
    return out, res


def kernel(**inputs) -> np.ndarray:
    out, _ = _run(inputs, trace=False)
    return out
